# revision 33
# baseline (speedup 1.0000x reference)
"""Trainium2 Bass kernel for nn_CriterionLP (LP contrastive criterion loss).

Reference computation (B=2048 anchors, M=16384 supports, C=256, K=128 label
blocks of G=128 supports each):
    sim   = (feats @ Fs.T) / TEMP                  [B, M]
    E     = exp(sim) grouped into K blocks of G    [B, K, G]
    pos   = exp(min sim over own-label block)      (one block per row)
    neg   = sum over other blocks of exp(max sim over block)
    loss  = mean_b( -log(pos/(pos+neg+eps) + eps) )

v17: ANCHOR sharding, zero on-device collectives (baseline v12 spent ~55us
in the collective path: 33us entry barrier + warm-up AR + 8KB AR).
  Each core owns 256 anchor rows (2 partition tiles) and streams the FULL
  fp8 support set (4MB, 16 chunks over the scalar/gpsimd/sync DMA queues,
  assigned in consumption order; the sync hw queue is ~3x slower so it
  only carries two mid-stream chunks + masks).  Every per-row quantity is
  computed locally; the host just sums the 8 per-core partial losses
  (pure gather/unshard of the all-reduce mean).

  Per-core pipeline, paced by the two PSUM-capable engines:
    - 64 fp8 DoubleRow matmuls [128,512], grouped as [128,2048] PSUM
      tiles; loop is chunk-outer/anchor-tile-inner so each fs chunk feeds
      both anchor tiles back-to-back (halves the early DMA demand rate,
      removing DMA stalls from the scalar copy stream)
    - scalar is the sole PSUM drainer: ACT Exp(scale=1/TEMP) f32->bf16
      into SBUF (E domain, exp folded into the copy), ~1.85us/group,
      back-to-back for the whole kernel
    - vector reduces the bf16 copies with a batched two-bt max tree
      (TT L1 + TT L2 + segmented TENSOR_REDUCE; TT gets the 2x packed
      mode, TENSOR_REDUCE is 1x-only on this hardware) plus min trees on
      the own-label group (host-side per-core block permutation parks all
      own-label blocks in group OWN_CH)
    - the final group of both anchor tiles is drained by direct
      tensor_reduce off PSUM on vector to shorten the tail
    - loss tail: STT accum for pos/neg, reciprocal, Ln, ones-matmul
      partition sum, single [1,1] DMA out.
"""

import numpy as np
import ml_dtypes

import concourse.bass as bass
import concourse.bacc as bacc
import concourse.tile as tile
import concourse.mybir as mybir
from concourse.bass_utils import run_bass_kernel_spmd

VERSION_TAG = "v17"

F32 = mybir.dt.float32
BF16 = mybir.dt.bfloat16
FP8 = mybir.dt.float8e4
AX = mybir.AxisListType
ALU = mybir.AluOpType
ACTF = mybir.ActivationFunctionType
DR = mybir.MatmulPerfMode.DoubleRow

TEMP = 0.05
EPS = 1e-6
B, C = 2048, 256
M = 16384                   # total support rows
NCORES = 8
BLOC = B // NCORES          # anchor rows per core (256)
NBT = BLOC // 128           # anchor tiles per core (2)
KTOT, G = 128, 128          # label blocks, supports per block
NCH = 16                    # fs DMA chunks of 1024 cols (8 blocks each)
KPC = 8                     # blocks per chunk
NGR = 8                     # PSUM drain groups per bt (2048 cols each)
KPG = 16                    # blocks per drain group
OWN_CH = 4                  # drain group holding own-label blocks (after perm)

# drain schedule per (bt, g) group of [128, 2048] PSUM (16 blocks):
# scalar is the sole PSUM drainer (ACT Exp scale=1/TEMP, f32 PSUM -> bf16
# SBUF in E domain, 1.97us/group); vector runs a short max tree on the bf16
# copy (TT L1 + TT L2 + segmented TR over 32, ~1.7us/group) plus min trees
# for the own-label group.  The last group per core is vector-direct
# (tensor_reduce off PSUM) to shorten the tail and shave a scalar copy.
# groups handled by vector tensor_reduce straight off PSUM (sim domain)
DIRECT_GROUPS = {(0, 0)}

_PROG_CACHE = {}
LAST_RESULT = None


def _build(fast):
    key = (fast, VERSION_TAG)
    if key in _PROG_CACHE:
        return _PROG_CACHE[key]

    pw = 8 if fast else KTOT    # pos-mask width

    nc = bacc.Bacc("TRN2", target_bir_lowering=False, debug=False,
                   num_devices=NCORES)
    ftd = nc.dram_tensor("ftq", [128, 2, BLOC], FP8, kind="ExternalInput")
    fsd = nc.dram_tensor("fsq", [128, NCH, 2, 1024], FP8, kind="ExternalInput")
    mnd = nc.dram_tensor("mneg", [128, NBT, KTOT], BF16, kind="ExternalInput")
    mpd = nc.dram_tensor("mpos", [128, NBT, pw], BF16, kind="ExternalInput")
    lossd = nc.dram_tensor("loss", [1, 1], F32, kind="ExternalOutput")

    with tile.TileContext(nc) as tc:
        with (
            tc.tile_pool(name="wpool", bufs=1) as wp,
            tc.tile_pool(name="cpool", bufs=3) as cpp,
            tc.tile_pool(name="tpool", bufs=2) as trp,
            tc.tile_pool(name="spool", bufs=1) as stp,
            tc.tile_pool(name="pspool", bufs=2, space="PSUM") as psp,
        ):
            # --- input loads, spread across the three DMA-capable queues.
            # All 8 cores pull ~34MB together, so queues run contention-bound
            # (~150 GB/s each, gpsimd software-DGE slower); chunks are
            # assigned round-robin in consumption (need) order.
            ft = wp.tile([128, 2, BLOC], FP8, name="ft", tag="ft")
            nc.gpsimd.dma_start(ft[:, :, :], ftd[:, :, :])
            fs = wp.tile([128, NCH, 2, 1024], FP8, name="fs", tag="fs")
            # fs[:, a:b] is contiguous per partition, so chunk runs go out
            # as single large DMAs.  The scalar queue gets only 4 trigger
            # instructions (trigger instrs occupy the engine queue and would
            # otherwise delay the first copy); run boundaries are chosen so
            # each run's completion semaphore lands before its first chunk
            # is consumed.  The sync engine is unused (its hw queue is ~3x
            # slower and its rendezvous spin-up is the preamble's long pole).
            for lo, hi in ((0, 1), (3, 5), (8, 10), (10, 13)):
                nc.scalar.dma_start(fs[:, lo:hi, :, :], fsd[:, lo:hi, :, :])
            for lo, hi in ((1, 2), (2, 3), (5, 7), (7, 8), (13, 16)):
                nc.gpsimd.dma_start(fs[:, lo:hi, :, :], fsd[:, lo:hi, :, :])
            mneg = stp.tile([128, NBT, KTOT], BF16, name="mneg_sb")
            nc.gpsimd.dma_start(mneg[:], mnd[:, :, :])
            mpos = stp.tile([128, NBT, pw], BF16, name="mpos_sb")
            nc.gpsimd.dma_start(mpos[:], mpd[:, :, :])

            # warm the Exp activation table while the first fs chunk is in
            # flight (table load is ~1.3us on the scalar queue)
            warm = stp.tile([128, 1], F32, name="warm")
            nc.vector.memset(warm[:], 0.0)
            warmo = stp.tile([128, 1], F32, name="warmo")
            nc.scalar.activation(warmo[:], warm[:], ACTF.Exp)

            # E-domain block stats per bt: E[p, bt, k] = exp(blockmax/TEMP)
            ebuf = stp.tile([128, NBT, KTOT], BF16, name="ebuf")
            minbuf = stp.tile([128, NBT, pw], BF16, name="minbuf")

            # g-outer / bt-inner: each fs chunk feeds both anchor tiles
            # back-to-back (halves the early DMA demand rate), and the two
            # bts' max trees batch into one TT/TT/TR sequence
            deferred = []
            for g in range(NGR):
                gsl = slice(g * KPG, (g + 1) * KPG)
                cp = cpp.tile([128, NBT, KPG, G], BF16, name="cp", tag="cp")
                direct = []
                for bt in range(NBT):
                    ps = psp.tile([128, 2048], F32, name="ps", tag="psg")
                    for sub in range(4):
                        nc.tensor.matmul(
                            ps[:, sub * 512:(sub + 1) * 512],
                            ft[:, :, bt * 128:(bt + 1) * 128],
                            fs[:, 2 * g + sub // 2, :,
                               (sub % 2) * 512:(sub % 2) * 512 + 512],
                            start=True, stop=True, perf_mode=DR,
                        )
                    if fast and (bt, g) in DIRECT_GROUPS:
                        # vector: segmented max off PSUM (sim domain); the
                        # tiny exp is deferred past the copy stream so it
                        # cannot block the scalar queue on the TR semaphore
                        sm = stp.tile([128, KPG], F32, name=f"sm_{bt}_{g}")
                        nc.vector.tensor_reduce(
                            sm[:], ps.rearrange("p (k g) -> p k g", g=G),
                            axis=AX.X, op=ALU.max)
                        deferred.append((sm, ebuf[:, bt, gsl]))
                        direct.append(bt)
                        continue
                    # scalar drains PSUM: exp(sim/TEMP) -> bf16 E copy
                    nc.scalar.activation(
                        cp[:, bt].rearrange("p k g -> p (k g)"), ps[:],
                        ACTF.Exp, scale=1.0 / TEMP)
                # batched max tree over both bts of this group
                if len(direct) < NBT:
                    if direct:
                        bts = [bt for bt in range(NBT) if bt not in direct]
                        cpv = cp[:, bts[0]:bts[0] + 1]
                        ev = ebuf[:, bts[0]:bts[0] + 1, gsl]
                        nb = 1
                    else:
                        cpv = cp[:, :]
                        ev = ebuf[:, :, gsl]
                        nb = NBT
                    t1 = trp.tile([128, nb, KPG, 64], BF16, name="t1",
                                  tag="t1", bufs=2)
                    nc.vector.tensor_tensor(t1[:], cpv[:, :, :, 0:64],
                                            cpv[:, :, :, 64:128], ALU.max)
                    t2 = trp.tile([128, nb, KPG, 32], BF16, name="t2",
                                  tag="t2", bufs=2)
                    nc.vector.tensor_tensor(t2[:], t1[:, :, :, 0:32],
                                            t1[:, :, :, 32:64], ALU.max)
                    t3 = trp.tile([128, nb, KPG, 16], BF16, name="t3",
                                  tag="t3", bufs=2)
                    nc.vector.tensor_tensor(t3[:], t2[:, :, :, 0:16],
                                            t2[:, :, :, 16:32], ALU.max)
                    t4 = trp.tile([128, nb, KPG, 8], BF16, name="t4",
                                  tag="t4", bufs=2)
                    nc.vector.tensor_tensor(t4[:], t3[:, :, :, 0:8],
                                            t3[:, :, :, 8:16], ALU.max)
                    nc.vector.tensor_reduce(ev, t4[:], axis=AX.X, op=ALU.max)
                if (g == OWN_CH) if fast else True:
                    for bt in range(NBT):
                        if fast:
                            cpv2 = cp[:, bt, 8 * bt:8 * bt + 8, :]
                            msl = minbuf[:, bt, :]
                            km = 8
                        else:
                            cpv2 = cp[:, bt]
                            msl = minbuf[:, bt, gsl]
                            km = KPG
                        m1 = trp.tile([128, km, 64], BF16, name="m1",
                                      tag="m1", bufs=2)
                        nc.vector.tensor_tensor(m1[:], cpv2[:, :, 0:64],
                                                cpv2[:, :, 64:128], ALU.min)
                        m2 = trp.tile([128, km, 32], BF16, name="m2",
                                      tag="m2", bufs=2)
                        nc.vector.tensor_tensor(m2[:], m1[:, :, 0:32],
                                                m1[:, :, 32:64], ALU.min)
                        nc.vector.tensor_reduce(msl, m2[:], axis=AX.X,
                                                op=ALU.min)

            for sm, esl2 in deferred:
                nc.scalar.activation(esl2, sm[:], ACTF.Exp, scale=1.0 / TEMP)

            # --- per-row pos/neg and loss ---
            neg = stp.tile([128, NBT], F32, name="neg")
            pos = stp.tile([128, NBT], F32, name="pos")
            nmask = stp.tile([128, NBT, KTOT], BF16, name="nmask")
            pmask = stp.tile([128, NBT, pw], BF16, name="pmask")
            for bt in range(NBT):
                nc.vector.scalar_tensor_tensor(
                    nmask[:, bt, :], ebuf[:, bt, :], 1.0, mneg[:, bt, :],
                    ALU.mult, ALU.mult, accum_out=neg[:, bt:bt + 1])
                nc.vector.scalar_tensor_tensor(
                    pmask[:, bt, :], minbuf[:, bt, :], 1.0, mpos[:, bt, :],
                    ALU.mult, ALU.mult, accum_out=pos[:, bt:bt + 1])

            den = stp.tile([128, NBT], F32, name="den")
            nc.vector.scalar_tensor_tensor(
                den[:], pos[:], float(EPS), neg[:], ALU.add, ALU.add)
            rec = stp.tile([128, NBT], F32, name="rec")
            nc.vector.reciprocal(rec[:], den[:])
            ratio = stp.tile([128, NBT], F32, name="ratio")
            nc.vector.tensor_mul(ratio[:], pos[:], rec[:])
            epsb = stp.tile([128, 1], F32, name="epsb")
            nc.vector.memset(epsb[:], float(EPS))
            lg = stp.tile([128, NBT], F32, name="lg")
            nc.scalar.activation(lg[:], ratio[:], ACTF.Ln, bias=epsb[:, 0:1])
            # partition sum via matmul; fold -1/B into the ones vector
            ones = stp.tile([128, 1], F32, name="ones")
            nc.vector.memset(ones[:], -1.0 / B)
            pl = psp.tile([128, 2048], F32, name="pl", tag="psg")
            nc.tensor.matmul(pl[:1, 0:NBT], ones[:], lg[:])
            lout = stp.tile([1, 1], F32, name="lout")
            nc.vector.tensor_reduce(lout[:], pl[:1, 0:NBT], axis=AX.X,
                                    op=ALU.add)
            nc.scalar.dma_start(lossd[:, :], lout[:])

    nc.compile()
    _PROG_CACHE[key] = nc
    return nc


def kernel(feats, feats_s, labels, labels_s, topk, num_instances):
    global LAST_RESULT
    feats = np.asarray(feats, dtype=np.float32)
    feats_s = np.asarray(feats_s, dtype=np.float32)
    labels = np.asarray(labels).astype(np.int64).ravel()
    labels_s = np.asarray(labels_s).astype(np.int64).ravel()
    tk, ni = int(topk), int(num_instances)
    assert tk * ni == G and feats.shape == (B, C)
    assert feats_s.shape == (B, tk, C)

    Fs = feats_s.reshape(-1, C)                       # [16384, 256]
    glab = labels_s.reshape(KTOT, G)[:, 0]            # label of each block

    # fast path requires the structured contiguous-cluster layout: core j's
    # 256 anchors own exactly blocks [16j, 16j+16), cluster c -> block c
    fast = bool(
        np.array_equal(labels, np.repeat(np.arange(KTOT), B // KTOT))
        and np.array_equal(glab, np.arange(KTOT))
        and np.array_equal(labels_s, np.repeat(labels, tk))
    )

    nc = _build(fast)
    pw = 8 if fast else KTOT

    in_maps = []
    for j in range(NCORES):
        rows = slice(j * BLOC, (j + 1) * BLOC)
        f_loc = feats[rows]                           # [256, 256]
        lab_loc = labels[rows]
        if fast:
            # permute blocks so core j's own 16 blocks sit at chunk OWN_CH
            own = list(range(16 * j, 16 * j + 16))
            others = [k for k in range(KTOT) if k not in own]
            order = others[:OWN_CH * KPG] + own + others[OWN_CH * KPG:]
        else:
            order = list(range(KTOT))
        order = np.asarray(order)
        Fs_perm = Fs.reshape(KTOT, G, C)[order].reshape(M, C)
        gl_perm = glab[order]

        ftq = np.ascontiguousarray(
            f_loc.T.reshape(2, 128, BLOC).transpose(1, 0, 2))
        fsq = np.ascontiguousarray(
            Fs_perm.T.reshape(2, 128, NCH, 1024).transpose(1, 2, 0, 3))

        # masks in [partition, bt, block] coords
        lab2 = lab_loc.reshape(NBT, 128).T            # [p, bt]
        mp_full = (lab2[:, :, None] == gl_perm[None, None, :])  # [p, bt, k]
        mn = (~mp_full).astype(ml_dtypes.bfloat16)
        if fast:
            # own block position is OWN_CH*16 + 8*bt + p//16; pos-select
            # mask over the 8 own-region blocks of this bt's min tree
            mp = np.zeros((128, NBT, 8), dtype=bool)
            for bt_i in range(NBT):
                own_pos = OWN_CH * KPG + 8 * bt_i + np.arange(128) // 16
                chk = np.zeros((128, KTOT), dtype=bool)
                chk[np.arange(128), own_pos] = True
                assert np.array_equal(chk, mp_full[:, bt_i, :]), "perm bug"
                mp[np.arange(128), bt_i, np.arange(128) // 16] = True
            mp = mp.astype(ml_dtypes.bfloat16)
        else:
            mp = mp_full.astype(ml_dtypes.bfloat16)

        in_maps.append({
            "ftq": ftq.astype(ml_dtypes.float8_e4m3),
            "fsq": fsq.astype(ml_dtypes.float8_e4m3),
            "mneg": mn,
            "mpos": mp,
        })

    LAST_RESULT = run_bass_kernel_spmd(nc, in_maps, core_ids=list(range(NCORES)))
    # host-side unshard: each core's [1,1] is sum(-log(...)/B) over its own
    # 256 anchor rows; the full-batch mean is the plain sum of the 8 shards
    total = 0.0
    for r in LAST_RESULT.results:
        total += float(np.asarray(r["loss"]).reshape(()))
    return np.float32(total)
